# revision 1
# baseline (speedup 1.0000x reference)
"""BYOL loss kernel for Trainium2 (8 NeuronCores, SPMD data-parallel).

loss = 2 - 2 * mean_n( <x_n, t_n> / (||x_n|| * ||t_n||) )   over N=8192 rows, D=512.

Sharding: rows split 1024/core across 8 cores. Each core computes the per-row
cosine for its shard (DVE fused multiply-reduce for the dot, ScalarE
square-with-accumulate for the norms); host gathers the 8192 per-row cosines
and takes the mean (the trivial "all-reduce" step).
"""

import sys

for _p in ("/opt/trn_rl_repo",):
    if _p not in sys.path:
        sys.path.insert(0, _p)

import numpy as np

import concourse.tile as tile
from concourse import bacc, mybir
from concourse import bass_utils

N, D = 8192, 512
NCORES = 8
N_LOC = N // NCORES          # 1024 rows per core
P = 128                      # partitions
NT = N_LOC // P              # 8 row-blocks of [128, 512] per core
CHUNKS = [1, 1, 2, 2, 1, 1]  # row-blocks per dma_start (small first = early start,
                             # small last = short pipeline drain)
IN_BUFS = len(CHUNKS)        # all chunks in flight: DMA ring never starves
SQUARE_ON_DVE = {(1, 2)}     # (tensor_idx, block) square-sums moved ACT -> DVE
                             # to balance engines (ACT 15 ops ~12.0us, DVE ~11.6us)
USE_BF16 = False             # cast inputs f32->bf16 during DMA (SWDGE): DVE muls 2x
                             # but ACT squares don't speed up and SWDGE adds ~2.5us
                             # startup latency — measured slower than HWDGE f32
T_ON_ACT_RING = False        # t-loads from ACT sequencer (qActDynamicHW): crashes
                             # the device in this stack — keep False

F32 = mybir.dt.float32
BF16 = mybir.dt.bfloat16


def _build():
    nc = bacc.Bacc("TRN2", target_bir_lowering=False, debug=False, num_devices=NCORES)
    x = nc.dram_tensor("x", [N_LOC, D], F32, kind="ExternalInput").ap()
    t = nc.dram_tensor("t", [N_LOC, D], F32, kind="ExternalInput").ap()
    # per-row dots: [:, 0:NT] = <x,t>, [:, NT:2NT] = <x,x>, [:, 2NT:3NT] = <t,t>
    out = nc.dram_tensor("dots", [P, 3 * NT], F32, kind="ExternalOutput").ap()

    # row r = i*128 + p  ->  tile index i, partition p
    xr = x.rearrange("(t p) d -> p t d", p=P)
    tr = t.rearrange("(t p) d -> p t d", p=P)

    assert sum(CHUNKS) == NT

    with tile.TileContext(nc) as tc:
        with (
            tc.tile_pool(name="xin", bufs=IN_BUFS) as xpool,
            tc.tile_pool(name="tin", bufs=IN_BUFS) as tpool,
            tc.tile_pool(name="scratch", bufs=4) as spool,
            tc.tile_pool(name="stats", bufs=1) as stats,
        ):
            dots = stats.tile([P, 3 * NT], F32, tag="dots")
            xt_s = dots[:, 0:NT]
            xx_s = dots[:, NT : 2 * NT]
            tt_s = dots[:, 2 * NT : 3 * NT]

            in_dt = BF16 if USE_BF16 else F32
            # t-loads on the second HWDGE ring (ACT sequencer issues them):
            # two FIFO rings interleave across the 16 SDMA engines
            t_dma_engine = nc.scalar if T_ON_ACT_RING else nc.sync

            def square_sum(tensor_idx, block, src_ap, acc_ap):
                if (tensor_idx, block) in SQUARE_ON_DVE:
                    sq = spool.tile([P, D], in_dt, tag="prod")
                    nc.vector.tensor_mul(sq[:], src_ap, src_ap)
                    nc.vector.reduce_sum(acc_ap, sq[:], axis=mybir.AxisListType.X)
                else:
                    sq = spool.tile([P, D], in_dt, tag="sq")
                    nc.scalar.activation(
                        sq[:], src_ap, mybir.ActivationFunctionType.Square,
                        accum_out=acc_ap,
                    )

            base = 0
            for sz in CHUNKS:
                xt_in = xpool.tile([P, sz, D], in_dt, tag="xd")
                tt_in = tpool.tile([P, sz, D], in_dt, tag="td")
                if USE_BF16:
                    nc.gpsimd.dma_start(xt_in[:], xr[:, base : base + sz, :])
                    nc.gpsimd.dma_start(tt_in[:], tr[:, base : base + sz, :])
                else:
                    nc.sync.dma_start(xt_in[:], xr[:, base : base + sz, :])
                    t_dma_engine.dma_start(tt_in[:], tr[:, base : base + sz, :])
                for j in range(sz):
                    i = base + j
                    xa = xt_in[:, j, :]
                    ta = tt_in[:, j, :]
                    square_sum(0, i, xa, xx_s[:, i : i + 1])
                    square_sum(1, i, ta, tt_s[:, i : i + 1])
                # <x, t>: one VectorE multiply + one free-dim reduce per chunk
                prod = spool.tile([P, sz, D], in_dt, tag="prod")
                nc.vector.tensor_mul(prod[:], xt_in[:], tt_in[:])
                nc.vector.reduce_sum(
                    xt_s[:, base : base + sz], prod[:], axis=mybir.AxisListType.X
                )
                base += sz

            nc.sync.dma_start(out, dots[:])

    nc.finalize()
    return nc


_nc_cache = None


def _get_nc():
    global _nc_cache
    if _nc_cache is None:
        _nc_cache = _build()
    return _nc_cache


def run(x, x_target, **spmd_kwargs):
    """Run the SPMD kernel; returns (loss, BassKernelResults)."""
    x = np.ascontiguousarray(np.asarray(x, dtype=np.float32))
    t = np.ascontiguousarray(np.asarray(x_target, dtype=np.float32))
    assert x.shape == (N, D) and t.shape == (N, D)
    nc = _get_nc()
    in_maps = [
        {
            "x": x[c * N_LOC : (c + 1) * N_LOC],
            "t": t[c * N_LOC : (c + 1) * N_LOC],
        }
        for c in range(NCORES)
    ]
    res = bass_utils.run_bass_kernel_spmd(
        nc, in_maps, core_ids=list(range(NCORES)), **spmd_kwargs
    )
    dots = np.stack([np.asarray(r["dots"]) for r in res.results]).astype(np.float64)
    xt = dots[:, :, 0:NT]
    xx = dots[:, :, NT : 2 * NT]
    tt = dots[:, :, 2 * NT : 3 * NT]
    EPS = 1e-8  # matches reference: a / max(||a||, eps) per tensor
    cos = xt / (np.maximum(np.sqrt(xx), EPS) * np.maximum(np.sqrt(tt), EPS))
    loss = 2.0 - 2.0 * float(np.mean(cos))
    return np.float32(loss), res


def kernel(x, x_target):
    loss, _ = run(x, x_target)
    return loss



# revision 11
# speedup vs baseline: 1.0185x; 1.0185x over previous
"""BYOL loss kernel for Trainium2 (8 NeuronCores, SPMD data-parallel).

loss = 2 - 2 * mean_n( <x_n, t_n> / (||x_n|| * ||t_n||) )   over N=8192 rows, D=512.

Sharding: rows split 1024/core across 8 cores. Each core computes the per-row
cosine for its shard (DVE fused multiply-reduce for the dot, ScalarE
square-with-accumulate for the norms); host gathers the 8192 per-row cosines
and takes the mean (the trivial "all-reduce" step).
"""

import sys

for _p in ("/opt/trn_rl_repo",):
    if _p not in sys.path:
        sys.path.insert(0, _p)

import numpy as np

import concourse.tile as tile
from concourse import bacc, mybir
from concourse import bass_utils

N, D = 8192, 512
NCORES = 8
N_LOC = N // NCORES          # 1024 rows per core
P = 128                      # partitions
NT = N_LOC // P              # 8 row-blocks of [128, 512] per core
CHUNKS = [1, 1, 2, 2, 1, 1]  # row-blocks per dma_start (small first = early start,
                             # small last = short pipeline drain)
IN_BUFS = len(CHUNKS)        # all chunks in flight: DMA ring never starves
SQUARE_ON_DVE = {(1, 2)}     # (tensor_idx, block) square-sums moved ACT -> DVE
                             # to balance engines (ACT 15 ops ~12.0us, DVE ~11.6us)

F32 = mybir.dt.float32
BF16 = mybir.dt.bfloat16


def _build():
    nc = bacc.Bacc("TRN2", target_bir_lowering=False, debug=False, num_devices=NCORES)
    x = nc.dram_tensor("x", [N_LOC, D], F32, kind="ExternalInput").ap()
    t = nc.dram_tensor("t", [N_LOC, D], F32, kind="ExternalInput").ap()
    # per-row dots: [:, 0:NT] = <x,t>, [:, NT:2NT] = <x,x>, [:, 2NT:3NT] = <t,t>
    out = nc.dram_tensor("dots", [P, 3 * NT], F32, kind="ExternalOutput").ap()

    # row r = i*128 + p  ->  tile index i, partition p
    xr = x.rearrange("(t p) d -> p t d", p=P)
    tr = t.rearrange("(t p) d -> p t d", p=P)

    assert sum(CHUNKS) == NT

    with tile.TileContext(nc) as tc:
        with (
            tc.tile_pool(name="xin", bufs=IN_BUFS) as xpool,
            tc.tile_pool(name="tin", bufs=IN_BUFS) as tpool,
            tc.tile_pool(name="scratch", bufs=4) as spool,
            tc.tile_pool(name="stats", bufs=1) as stats,
        ):
            dots = stats.tile([P, 3 * NT], F32, tag="dots")
            xt_s = dots[:, 0:NT]
            xx_s = dots[:, NT : 2 * NT]
            tt_s = dots[:, 2 * NT : 3 * NT]

            in_dt = F32

            def square_sum(tensor_idx, block, src_ap, acc_ap):
                if (tensor_idx, block) in SQUARE_ON_DVE:
                    sq = spool.tile([P, D], in_dt, tag="prod")
                    nc.vector.tensor_mul(sq[:], src_ap, src_ap)
                    nc.vector.reduce_sum(acc_ap, sq[:], axis=mybir.AxisListType.X)
                else:
                    sq = spool.tile([P, D], in_dt, tag="sq")
                    nc.scalar.activation(
                        sq[:], src_ap, mybir.ActivationFunctionType.Square,
                        accum_out=acc_ap,
                    )

            base = 0
            for sz in CHUNKS:
                xt_in = xpool.tile([P, sz, D], in_dt, tag="xd")
                tt_in = tpool.tile([P, sz, D], in_dt, tag="td")
                nc.sync.dma_start(xt_in[:], xr[:, base : base + sz, :])
                nc.sync.dma_start(tt_in[:], tr[:, base : base + sz, :])
                for j in range(sz):
                    i = base + j
                    xa = xt_in[:, j, :]
                    ta = tt_in[:, j, :]
                    square_sum(0, i, xa, xx_s[:, i : i + 1])
                    square_sum(1, i, ta, tt_s[:, i : i + 1])
                # <x, t>: one VectorE multiply + one free-dim reduce per chunk
                prod = spool.tile([P, sz, D], in_dt, tag="prod")
                nc.vector.tensor_mul(prod[:], xt_in[:], tt_in[:])
                nc.vector.reduce_sum(
                    xt_s[:, base : base + sz], prod[:], axis=mybir.AxisListType.X
                )
                base += sz

            nc.sync.dma_start(out, dots[:])

    nc.finalize()
    return nc


_nc_cache = None


def _get_nc():
    global _nc_cache
    if _nc_cache is None:
        _nc_cache = _build()
    return _nc_cache


def run(x, x_target, **spmd_kwargs):
    """Run the SPMD kernel; returns (loss, BassKernelResults)."""
    x = np.ascontiguousarray(np.asarray(x, dtype=np.float32))
    t = np.ascontiguousarray(np.asarray(x_target, dtype=np.float32))
    assert x.shape == (N, D) and t.shape == (N, D)
    nc = _get_nc()
    in_maps = [
        {
            "x": x[c * N_LOC : (c + 1) * N_LOC],
            "t": t[c * N_LOC : (c + 1) * N_LOC],
        }
        for c in range(NCORES)
    ]
    res = bass_utils.run_bass_kernel_spmd(
        nc, in_maps, core_ids=list(range(NCORES)), **spmd_kwargs
    )
    dots = np.stack([np.asarray(r["dots"]) for r in res.results]).astype(np.float64)
    xt = dots[:, :, 0:NT]
    xx = dots[:, :, NT : 2 * NT]
    tt = dots[:, :, 2 * NT : 3 * NT]
    EPS = 1e-8  # matches reference: a / max(||a||, eps) per tensor
    cos = xt / (np.maximum(np.sqrt(xx), EPS) * np.maximum(np.sqrt(tt), EPS))
    loss = 2.0 - 2.0 * float(np.mean(cos))
    return np.float32(loss), res


def kernel(x, x_target):
    loss, _ = run(x, x_target)
    return loss


# revision 12
# speedup vs baseline: 1.1531x; 1.1321x over previous
"""BYOL loss kernel for Trainium2 (8 NeuronCores, SPMD data-parallel).

loss = 2 - 2 * mean_n( <x_n, t_n> / (||x_n|| * ||t_n||) )   over N=8192 rows, D=512.

Sharding: rows split 1024/core across 8 cores. Each core computes 24 per-row
block reductions ([128,512] -> [128,1]): 8 x.t dots, 8 ||x||^2, 8 ||t||^2.
v3: single-pass fused reduce ops split over DVE + ACT so compute keeps pace
with the HWDGE input stream (which already runs at fabric line rate ~430GB/s):
  - DVE: affine_mul_reduce (out=(a*b), accum=sum), ~0.73us/block: 8 xt + 6 tt
  - ACT: activation(Square, accum_out), ~1.09us/block: 8 xx + 2 tt
(GpSimd can't help: TensorScalarPtr is ISA-rejected on Pool and Pool has no
free-axis reduce, so a Pool multiply still needs a DVE reduce that costs DVE
more than the whole fused block.)
Host gathers the 8192 per-row stats and takes the mean (trivial all-reduce).
"""

import sys

for _p in ("/opt/trn_rl_repo",):
    if _p not in sys.path:
        sys.path.insert(0, _p)

import numpy as np

import concourse.tile as tile
from concourse import bacc, mybir
from concourse import bass_utils

N, D = 8192, 512
NCORES = 8
N_LOC = N // NCORES          # 1024 rows per core
P = 128                      # partitions
NT = N_LOC // P              # 8 row-blocks of [128, 512] per core
CHUNKS = [1, 1, 2, 2, 1, 1]  # row-blocks per dma_start (small first = early start,
                             # small last = short pipeline drain)
IN_BUFS = len(CHUNKS)        # all chunks in flight: DMA ring never starves
TT_ON_ACT = 2                # tt blocks 0..1 on ACT (squares), 2..7 on DVE:
                             # DVE 14 x 0.73us = 10.2, ACT 10 x 1.09us = 10.9

F32 = mybir.dt.float32


def _build():
    nc = bacc.Bacc("TRN2", target_bir_lowering=False, debug=False, num_devices=NCORES)
    x = nc.dram_tensor("x", [N_LOC, D], F32, kind="ExternalInput").ap()
    t = nc.dram_tensor("t", [N_LOC, D], F32, kind="ExternalInput").ap()
    # per-row dots: [:, 0:NT] = <x,t>, [:, NT:2NT] = <x,x>, [:, 2NT:3NT] = <t,t>
    out = nc.dram_tensor("dots", [P, 3 * NT], F32, kind="ExternalOutput").ap()

    # row r = i*128 + p  ->  tile index i, partition p
    xr = x.rearrange("(t p) d -> p t d", p=P)
    tr = t.rearrange("(t p) d -> p t d", p=P)

    assert sum(CHUNKS) == NT

    with tile.TileContext(nc) as tc:
        with (
            tc.tile_pool(name="xin", bufs=IN_BUFS) as xpool,
            tc.tile_pool(name="tin", bufs=IN_BUFS) as tpool,
            tc.tile_pool(name="scratch", bufs=4) as spool,
            tc.tile_pool(name="stats", bufs=1) as stats,
        ):
            dots = stats.tile([P, 3 * NT], F32, tag="dots")
            xt_s = dots[:, 0:NT]
            xx_s = dots[:, NT : 2 * NT]
            tt_s = dots[:, 2 * NT : 3 * NT]

            def dve_dot(a, b, acc):
                # affine_mul_reduce: out=(a*1+0)*b, accum=sum — single DVE
                # pass; this custom-DVE op is exercised by tile_groupnorm_bwd
                # on real HW (tensor_tensor_reduce hangs this stack).
                prod = spool.tile([P, D], F32, tag="dve_prod")
                nc.vector.affine_mul_reduce(
                    out=prod[:], accum_out=acc, in0=a, in1=b,
                    scale=1.0, bias=0.0,
                )

            base = 0
            for sz in CHUNKS:
                xt_in = xpool.tile([P, sz, D], F32, tag="xd")
                tt_in = tpool.tile([P, sz, D], F32, tag="td")
                nc.sync.dma_start(xt_in[:], xr[:, base : base + sz, :])
                nc.sync.dma_start(tt_in[:], tr[:, base : base + sz, :])
                for j in range(sz):
                    i = base + j
                    xa = xt_in[:, j, :]
                    ta = tt_in[:, j, :]
                    # <x,t> on DVE (fused multiply+row-sum, single pass)
                    dve_dot(xa, ta, xt_s[:, i : i + 1])
                    # ||x||^2 on ACT (square with internal accumulator)
                    sq = spool.tile([P, D], F32, tag="sq")
                    nc.scalar.activation(
                        sq[:], xa, mybir.ActivationFunctionType.Square,
                        accum_out=xx_s[:, i : i + 1],
                    )
                    # ||t||^2: ACT squares for the first blocks, DVE fused rest
                    if i < TT_ON_ACT:
                        sqt = spool.tile([P, D], F32, tag="sqt")
                        nc.scalar.activation(
                            sqt[:], ta, mybir.ActivationFunctionType.Square,
                            accum_out=tt_s[:, i : i + 1],
                        )
                    else:
                        dve_dot(ta, ta, tt_s[:, i : i + 1])
                base += sz

            nc.sync.dma_start(out, dots[:])

    nc.finalize()
    return nc


_nc_cache = None


def _get_nc():
    global _nc_cache
    if _nc_cache is None:
        _nc_cache = _build()
    return _nc_cache


def run(x, x_target, **spmd_kwargs):
    """Run the SPMD kernel; returns (loss, BassKernelResults)."""
    x = np.ascontiguousarray(np.asarray(x, dtype=np.float32))
    t = np.ascontiguousarray(np.asarray(x_target, dtype=np.float32))
    assert x.shape == (N, D) and t.shape == (N, D)
    nc = _get_nc()
    in_maps = [
        {
            "x": x[c * N_LOC : (c + 1) * N_LOC],
            "t": t[c * N_LOC : (c + 1) * N_LOC],
        }
        for c in range(NCORES)
    ]
    res = bass_utils.run_bass_kernel_spmd(
        nc, in_maps, core_ids=list(range(NCORES)), **spmd_kwargs
    )
    dots = np.stack([np.asarray(r["dots"]) for r in res.results]).astype(np.float64)
    xt = dots[:, :, 0:NT]
    xx = dots[:, :, NT : 2 * NT]
    tt = dots[:, :, 2 * NT : 3 * NT]
    EPS = 1e-8  # matches reference: a / max(||a||, eps) per tensor
    cos = xt / (np.maximum(np.sqrt(xx), EPS) * np.maximum(np.sqrt(tt), EPS))
    loss = 2.0 - 2.0 * float(np.mean(cos))
    return np.float32(loss), res


def kernel(x, x_target):
    loss, _ = run(x, x_target)
    return loss


# revision 13
# speedup vs baseline: 1.2217x; 1.0595x over previous
"""BYOL loss kernel for Trainium2 (8 NeuronCores, SPMD data-parallel).

loss = 2 - 2 * mean_n( <x_n, t_n> / (||x_n|| * ||t_n||) )   over N=8192 rows, D=512.

Sharding: rows split 1024/core across 8 cores. Each core computes 24 per-row
block reductions ([128,512] -> [128,1]): 8 x.t dots, 8 ||x||^2, 8 ||t||^2.

The HWDGE input stream runs at ~270-290 GB/s under 8-core HBM contention
(~15us for 4MiB), so the kernel is stream-bound; compute must merely keep
pace and finish the last chunk fast:
  - DVE: affine_mul_reduce (out=(a*b), accum=sum, single pass ~0.61us/block):
         8 xt dots + tt blocks {2,3,4,7}
  - ACT: activation(Square, accum_out) ~0.8us/block: 8 xx + tt {0,1,5,6}
  - last chunk carries only xt_7+tt_7 (DVE) and xx_7 (ACT) -> ~1.2us tail
  - dots stored block-major [P, NT, 3]; blocks 0..6 are DMA'd out early so
    only block 7's 12B/partition ride the final DMA receipt (~2us)
(GpSimd compute is out: TensorScalarPtr is ISA-rejected on Pool and Pool has
no free-axis reduce. tensor_tensor_reduce on DVE hangs this HW stack; the
custom-DVE affine_mul_reduce is production-proven.)
Host gathers the 8192 per-row stats and takes the mean (trivial all-reduce).
"""

import sys

for _p in ("/opt/trn_rl_repo",):
    if _p not in sys.path:
        sys.path.insert(0, _p)

import numpy as np

import concourse.tile as tile
from concourse import bacc, mybir
from concourse import bass_utils

N, D = 8192, 512
NCORES = 8
N_LOC = N // NCORES          # 1024 rows per core
P = 128                      # partitions
NT = N_LOC // P              # 8 row-blocks of [128, 512] per core
CHUNKS = [1, 1, 2, 2, 1, 1]  # row-blocks per dma_start (small first = early start,
                             # small last = short pipeline drain)
IN_BUFS = len(CHUNKS)        # all chunks in flight: DMA ring never starves
TT_ON_ACT = {0, 1, 5, 6}     # tt blocks on ACT; rest fused on DVE

F32 = mybir.dt.float32


def _build():
    nc = bacc.Bacc("TRN2", target_bir_lowering=False, debug=False, num_devices=NCORES)
    x = nc.dram_tensor("x", [N_LOC, D], F32, kind="ExternalInput").ap()
    t = nc.dram_tensor("t", [N_LOC, D], F32, kind="ExternalInput").ap()
    # block-major per-row stats: dots[p, i, :] = (<x,t>, <x,x>, <t,t>) of block i
    out = nc.dram_tensor("dots", [P, NT, 3], F32, kind="ExternalOutput").ap()

    # row r = i*128 + p  ->  tile index i, partition p
    xr = x.rearrange("(t p) d -> p t d", p=P)
    tr = t.rearrange("(t p) d -> p t d", p=P)

    assert sum(CHUNKS) == NT

    with tile.TileContext(nc) as tc:
        with (
            tc.tile_pool(name="xin", bufs=IN_BUFS) as xpool,
            tc.tile_pool(name="tin", bufs=IN_BUFS) as tpool,
            tc.tile_pool(name="scratch", bufs=4) as spool,
            tc.tile_pool(name="stats", bufs=1) as stats,
        ):
            dots = stats.tile([P, NT, 3], F32, tag="dots")

            def dve_dot(a, b, acc):
                # single-pass fused multiply + row-sum (custom DVE op,
                # exercised on HW by tile_groupnorm_bwd)
                prod = spool.tile([P, D], F32, tag="dve_prod")
                nc.vector.affine_mul_reduce(
                    out=prod[:], accum_out=acc, in0=a, in1=b,
                    scale=1.0, bias=0.0,
                )

            def act_square(a, acc):
                sq = spool.tile([P, D], F32, tag="sq")
                nc.scalar.activation(
                    sq[:], a, mybir.ActivationFunctionType.Square,
                    accum_out=acc,
                )

            base = 0
            for sz in CHUNKS:
                xt_in = xpool.tile([P, sz, D], F32, tag="xd")
                tt_in = tpool.tile([P, sz, D], F32, tag="td")
                nc.sync.dma_start(xt_in[:], xr[:, base : base + sz, :])
                nc.sync.dma_start(tt_in[:], tr[:, base : base + sz, :])
                for j in range(sz):
                    i = base + j
                    xa = xt_in[:, j, :]
                    ta = tt_in[:, j, :]
                    dve_dot(xa, ta, dots[:, i, 0:1])        # <x,t> on DVE
                    act_square(xa, dots[:, i, 1:2])         # ||x||^2 on ACT
                    if i in TT_ON_ACT:                      # ||t||^2
                        act_square(ta, dots[:, i, 2:3])
                    else:
                        dve_dot(ta, ta, dots[:, i, 2:3])
                base += sz

            # early out-DMA for blocks 0..6; only block 7's stats ride the
            # final DMA's ~2us completion receipt
            nc.sync.dma_start(out[:, 0 : NT - 1, :], dots[:, 0 : NT - 1, :])
            nc.sync.dma_start(out[:, NT - 1 : NT, :], dots[:, NT - 1 : NT, :])

    nc.finalize()
    return nc


_nc_cache = None


def _get_nc():
    global _nc_cache
    if _nc_cache is None:
        _nc_cache = _build()
    return _nc_cache


def run(x, x_target, **spmd_kwargs):
    """Run the SPMD kernel; returns (loss, BassKernelResults)."""
    x = np.ascontiguousarray(np.asarray(x, dtype=np.float32))
    t = np.ascontiguousarray(np.asarray(x_target, dtype=np.float32))
    assert x.shape == (N, D) and t.shape == (N, D)
    nc = _get_nc()
    in_maps = [
        {
            "x": x[c * N_LOC : (c + 1) * N_LOC],
            "t": t[c * N_LOC : (c + 1) * N_LOC],
        }
        for c in range(NCORES)
    ]
    res = bass_utils.run_bass_kernel_spmd(
        nc, in_maps, core_ids=list(range(NCORES)), **spmd_kwargs
    )
    dots = np.stack([np.asarray(r["dots"]) for r in res.results]).astype(np.float64)
    xt = dots[:, :, :, 0]
    xx = dots[:, :, :, 1]
    tt = dots[:, :, :, 2]
    EPS = 1e-8  # matches reference: a / max(||a||, eps) per tensor
    cos = xt / (np.maximum(np.sqrt(xx), EPS) * np.maximum(np.sqrt(tt), EPS))
    loss = 2.0 - 2.0 * float(np.mean(cos))
    return np.float32(loss), res


def kernel(x, x_target):
    loss, _ = run(x, x_target)
    return loss
